# revision 3
# baseline (speedup 1.0000x reference)
"""MLA absorbed-QKVO attention kernel for Trainium2 (8 NeuronCores), v4.

Sharding: heads (H=16) tensor-parallel across 8 cores, 2 heads/core.
Host fuses W_h = w_qb_h @ w_qa (per-core), pre-transposes/casts to
bf16. Each core computes a partial output (its 2 heads through its w_o
column block); the host sums the 8 partials.

v4 vs v3 (baseline):
  - AV matmuls use V chunks as the stationary operand and stream the
    exp'd transposed scores, producing attnoutT (d-major) directly in
    PSUM -- no per-head PE transposes of the attention output.
  - The softmax denominator comes out as row 64 of the 5th AV psum
    (ones column of V); it is broadcast across partitions (gpsimd) and
    the reciprocal applied at attnoutT evacuation (DVE), so attnoutT
    is normalized on-device.
  - attnoutT is packed to 9x128 rows exactly (both heads, no pad), so
    the out GEMM does 9 accumulating matmuls per tile instead of 10.
  - V is assembled with DMA xbar transposes (sync/scalar HWDGE queues)
    instead of PE transposes.
  - QK/exp/AV restrict the streamed query range on diagonal key tiles
    (causality at 128 granularity).
  - hidT is double-buffered; one-time weight loads are ordered so the
    first consumers arrive first.
"""

import sys

import numpy as np

if "/opt/trn_rl_repo" not in sys.path:
    sys.path.insert(0, "/opt/trn_rl_repo")

import ml_dtypes

BF = ml_dtypes.bfloat16

B, S, HID = 2, 2048, 2048
H = 16
QK_ROPE = 64
KVR = 512
QLR = 1536
KVD = 640
DHEAD = 576
N_CORES = 8
HPC = H // N_CORES
OC = HPC * DHEAD      # 1152
SCALE = 1.0 / float(np.sqrt(128.0))

P = 128
SBLK = 512


def build_nc(b_count=B, s_len=S, debug=False, stage=3):
    import concourse.bass as bass  # noqa: F401
    import concourse.mybir as mybir
    import concourse.tile as tile
    from concourse import bacc

    fp32 = mybir.dt.float32
    bf16 = mybir.dt.bfloat16
    Exp = mybir.ActivationFunctionType.Exp

    NB = s_len // SBLK          # blocks per batch
    NKC = HID // P              # 16 hid chunks
    R = b_count * s_len

    nc = bacc.Bacc(None, target_bir_lowering=False)

    hidT_d = nc.dram_tensor("hidT", [HID, R], bf16, kind="ExternalInput")
    whT_d = nc.dram_tensor("whT", [HID, OC], bf16, kind="ExternalInput")
    wkvT_d = nc.dram_tensor("wkvT", [HID, KVD], bf16, kind="ExternalInput")
    woT_d = nc.dram_tensor("woT", [OC, HID], bf16, kind="ExternalInput")
    ropeT_d = nc.dram_tensor("ropeT", [P, s_len], bf16, kind="ExternalInput")
    maskd_d = nc.dram_tensor("maskd", [P, P], bf16, kind="ExternalInput")
    out_d = nc.dram_tensor("out_part", [R, HID], bf16, kind="ExternalOutput")
    if debug:
        dbg_key = nc.dram_tensor("dbg_key", [P, 5, s_len], bf16,
                                 kind="ExternalOutput")
        dbg_v = nc.dram_tensor("dbg_v", [P, s_len // P, 577], bf16,
                               kind="ExternalOutput")
        dbg_q = nc.dram_tensor("dbg_q", [P, 10, SBLK], bf16,
                               kind="ExternalOutput")
        dbg_at = nc.dram_tensor("dbg_at", [P, 9, SBLK], bf16,
                                kind="ExternalOutput")

    with tile.TileContext(nc) as tc:
        with (
            tc.tile_pool(name="singles", bufs=1) as singles,
            tc.tile_pool(name="batch", bufs=1) as batch,
            tc.tile_pool(name="work", bufs=1) as work,
            tc.tile_pool(name="strm", bufs=1) as strm,
            tc.tile_pool(name="stats", bufs=1) as stats,
            tc.tile_pool(name="psQ", bufs=3, space="PSUM") as psQ,
            tc.tile_pool(name="psA", bufs=5, space="PSUM") as psA,
        ):
            # ---- one-time loads, first consumer first ----
            # sync queue: wkvT -> maskd -> (per-blk hidT)
            # scalar queue: ropeT -> whT(a<8) -> whT(a>=8) -> woT
            wkvT = singles.tile([P, NKC, KVD], bf16, name="wkvT")
            nc.sync.dma_start(
                out=wkvT[:, 0:8, :],
                in_=wkvT_d.rearrange("(a p) m -> p a m", p=P)[:, 0:8, :])
            maskd = singles.tile([P, P], bf16, name="maskd")
            ropeT = singles.tile([P, s_len], bf16, name="ropeT")
            nc.scalar.dma_start(out=ropeT[:, :], in_=ropeT_d[:, :])
            whT = singles.tile([P, NKC, OC], bf16, name="whT")
            nc.scalar.dma_start(
                out=whT[:, 0:8, :],
                in_=whT_d.rearrange("(a p) m -> p a m", p=P)[:, 0:8, :])
            nc.scalar.dma_start(
                out=whT[:, 8:16, :],
                in_=whT_d.rearrange("(a p) m -> p a m", p=P)[:, 8:16, :])
            from concourse.masks import make_identity
            identb = singles.tile([P, P], bf16, name="identb")
            make_identity(nc, identb[:, :])
            woT = singles.tile([P, 9, HID], bf16, name="woT")
            nc.scalar.dma_start(
                out=woT[:, :, :],
                in_=woT_d.rearrange("(a p) m -> p a m", p=P))

            def rope_apply(dst_hi, dst_lo, src0, src32, cols):
                """dst rows <- rope(src [64 PSUM rows; src0=rows 0:32,
                src32=rows 32:64 at any partition base]).

                Table: rows 0:64 cos, 64:128 swizzled sin. m2 is written
                half-swapped so every SBUF+SBUF op below has equal input
                base partitions (a walrus verifier requirement).
                """
                m1 = strm.tile([64, SBLK], bf16, tag="m1", bufs=2, name="m1")
                m2 = strm.tile([64, SBLK], bf16, tag="m2", bufs=2, name="m2")
                nc.vector.tensor_mul(m1[0:32, :], src0, ropeT[0:32, cols])
                nc.vector.tensor_mul(m1[32:64, :], src32, ropeT[32:64, cols])
                nc.vector.tensor_mul(m2[32:64, :], src0, ropeT[64:96, cols])
                nc.vector.tensor_mul(m2[0:32, :], src32, ropeT[96:128, cols])
                nc.vector.tensor_sub(dst_hi, m1[0:32, :], m2[0:32, :])
                nc.vector.tensor_add(dst_lo, m1[32:64, :], m2[32:64, :])

            for b in range(b_count):
                # keyT slots: s0=[k_rope;nope0:64], s1..3=nope64:448,
                # s4 rows0:64=nope448:512
                keyT = batch.tile([P, 5, s_len], bf16, tag="keyT",
                                  name="keyT")
                # 592 (x2B = 1184, 32B-aligned) so DMA xbar transpose
                # dests land aligned for every kt; cols 577:592 unused
                V = batch.tile([P, s_len // P, 592], bf16, tag="V",
                               name="V")
                nc.gpsimd.memset(V[:, :, 576:577], 1.0)
                nc.gpsimd.memset(keyT[64:128, 4, :], 0.0)

                for blk in range(NB):
                    tok0 = blk * SBLK
                    rows0 = b * s_len + tok0
                    bcols = slice(tok0, tok0 + SBLK)

                    hidTa = work.tile([P, 8, SBLK], bf16, tag="hidTa",
                                      bufs=2, name="hidTa")
                    nc.sync.dma_start(
                        out=hidTa[:, :, :],
                        in_=hidT_d[0:8 * P, rows0:rows0 + SBLK].rearrange(
                            "(a p) s -> p a s", p=P))
                    if b == 0 and blk == 0:
                        nc.sync.dma_start(
                            out=wkvT[:, 8:16, :],
                            in_=wkvT_d.rearrange("(a p) m -> p a m",
                                                 p=P)[:, 8:16, :])
                    hidTb = work.tile([P, 8, SBLK], bf16, tag="hidTb",
                                      bufs=2, name="hidTb")
                    nc.sync.dma_start(
                        out=hidTb[:, :, :],
                        in_=hidT_d[8 * P:16 * P,
                                   rows0:rows0 + SBLK].rearrange(
                            "(a p) s -> p a s", p=P))
                    if b == 0 and blk == 0:
                        nc.sync.dma_start(out=maskd[:, :], in_=maskd_d[:, :])

                    def hid(a):
                        return hidTa[:, a, :] if a < 8 else hidTb[:, a - 8, :]

                    # ---- kv projection -> keyT slots (+rope) + vrope ----
                    vk0 = work.tile([P, SBLK], bf16, tag="vk0",
                                    bufs=2, name="vk0")
                    for c in range(5):
                        ps = psQ.tile([P, SBLK], fp32, tag="psQ", name="psQ")
                        for a in range(NKC):
                            nc.tensor.matmul(
                                ps[:, :], wkvT[:, a, c * P:(c + 1) * P],
                                hid(a),
                                start=(a == 0), stop=(a == NKC - 1))
                        if c == 0:
                            rope_apply(keyT[0:32, 0, bcols],
                                       keyT[32:64, 0, bcols],
                                       ps[0:32, :], ps[32:64, :], bcols)
                            nc.vector.tensor_copy(out=vk0[0:64, :],
                                                  in_=ps[64:128, :])
                        else:
                            nc.vector.tensor_copy(
                                out=keyT[64:128, c - 1, bcols],
                                in_=ps[0:64, :])
                            nc.scalar.copy(
                                out=keyT[0:64, c, bcols],
                                in_=ps[64:128, :])
                            if c == 1:
                                nc.scalar.copy(out=vk0[64:128, :],
                                               in_=ps[0:64, :])

                    # ---- fused q projection -> queryT slots (+rope) ----
                    queryT = work.tile([P, 10, SBLK], bf16, tag="queryT",
                                       bufs=1, name="queryT")
                    for c in range(9):
                        ps = psQ.tile([P, SBLK], fp32, tag="psQ", name="psQ")
                        for a in range(NKC):
                            nc.tensor.matmul(
                                ps[:, :], whT[:, a, c * P:(c + 1) * P],
                                hid(a),
                                start=(a == 0), stop=(a == NKC - 1))
                        if c == 0:
                            rope_apply(queryT[0:32, 0, :],
                                       queryT[32:64, 0, :],
                                       ps[0:32, :], ps[32:64, :], bcols)
                            nc.scalar.copy(out=queryT[64:128, 0, :],
                                           in_=ps[64:128, :])
                        elif c < 4:
                            nc.scalar.copy(out=queryT[:, c, :], in_=ps[:, :])
                        elif c == 4:
                            nc.scalar.copy(out=queryT[0:64, 4, :],
                                           in_=ps[0:64, :])
                            rope_apply(queryT[0:32, 5, :],
                                       queryT[32:64, 5, :],
                                       ps[64:96, :], ps[96:128, :], bcols)
                        else:
                            nc.vector.tensor_copy(
                                out=queryT[64:128, c, :], in_=ps[0:64, :])
                            nc.vector.tensor_copy(
                                out=queryT[0:64, c + 1, :], in_=ps[64:128, :])

                    # ---- V assembly via PE transposes (k-major);
                    # emitted after the q GEMM so keyT evacuations
                    # complete while the PE streams q matmuls ----
                    for sc in range(4):
                        tkc = blk * 4 + sc
                        kcols = slice(tok0 + sc * P, tok0 + (sc + 1) * P)
                        lcols = slice(sc * P, (sc + 1) * P)
                        tr = psQ.tile([P, 640], bf16, tag="psQ",
                                      name="trV")
                        nc.tensor.transpose(tr[:, 0:128], vk0[:, lcols],
                                            identb[:, :])
                        for c in range(1, 4):
                            nc.tensor.transpose(tr[:, c * P:(c + 1) * P],
                                                keyT[:, c, kcols],
                                                identb[:, :])
                        nc.tensor.transpose(tr[:, 512:640],
                                            keyT[:, 4, kcols],
                                            identb[:, :])
                        nc.scalar.copy(out=V[:, tkc, 0:576],
                                       in_=tr[:, 0:576])

                    # ---- attention (2 heads) -> normalized attnoutT ----
                    # attnoutT packing (9 slots x 128 rows = both heads):
                    #  s0..3 = h0 d0:512, s4 = [h0 d512:576; h1 d0:64],
                    #  s5..8 = h1 d64:576   (d-order = [v_rope, nope])
                    attnoutT = work.tile([P, 9, SBLK], bf16, tag="attnoutT",
                                         bufs=1, name="attnoutT")
                    nkt = (blk + 1) * 4
                    for hh in range(HPC if stage >= 2 else 0):
                        at = [psA.tile([P, SBLK], fp32, tag="psA",
                                       name="at") for _ in range(5)]

                        def q0_of(kt):
                            return (kt % 4) * P if kt // 4 == blk else 0

                        def qk(kt):
                            ps = psQ.tile([P, SBLK], fp32, tag="psQ",
                                          name="psS")
                            q0 = q0_of(kt)
                            for s_i in range(5):
                                kw = 64 if s_i == 4 else P
                                nc.tensor.matmul(
                                    ps[:, q0:SBLK],
                                    keyT[0:kw, s_i, kt * P:(kt + 1) * P],
                                    queryT[0:kw, hh * 5 + s_i, q0:SBLK],
                                    start=(s_i == 0), stop=(s_i == 4))
                            return ps

                        ps_cur = qk(0)
                        for kt in range(nkt):
                            ps_nxt = qk(kt + 1) if kt + 1 < nkt else None
                            q0 = q0_of(kt)
                            ept = strm.tile([P, SBLK], bf16, tag="ept",
                                            bufs=3, name="ept")
                            nc.scalar.activation(ept[:, q0:SBLK],
                                                 ps_cur[:, q0:SBLK],
                                                 Exp, scale=SCALE)
                            if kt // 4 == blk:
                                nc.vector.tensor_mul(
                                    ept[:, q0:q0 + P], ept[:, q0:q0 + P],
                                    maskd[:, 0:P])
                            st, sp = (kt == 0), (kt == nkt - 1)
                            for c in range(4):
                                nc.tensor.matmul(
                                    at[c][:, q0:SBLK],
                                    V[:, kt, c * P:(c + 1) * P],
                                    ept[:, q0:SBLK],
                                    start=st, stop=sp, skip_group_check=True)
                            nc.tensor.matmul(
                                at[4][0:65, q0:SBLK],
                                V[:, kt, 512:577],
                                ept[:, q0:SBLK],
                                start=st, stop=sp, skip_group_check=True)
                            ps_cur = ps_nxt

                        # denominator -> reciprocal, broadcast to all rows
                        den = stats.tile([1, SBLK], fp32, tag="den",
                                         name="den")
                        nc.vector.tensor_copy(out=den[0:1, :],
                                              in_=at[4][64:65, :])
                        rb = strm.tile([P, SBLK], fp32, tag="rb", bufs=2,
                                       name="rb")
                        nc.gpsimd.partition_broadcast(rb[:, :], den[0:1, :])
                        nc.vector.reciprocal(rb[:, :], rb[:, :])

                        # evacuate normalized attnoutT (packed)
                        if hh == 0:
                            for c in range(4):
                                nc.vector.tensor_mul(
                                    attnoutT[:, c, :], at[c][:, :], rb[:, :])
                            nc.vector.tensor_mul(
                                attnoutT[0:64, 4, :], at[4][0:64, :],
                                rb[0:64, :])
                        else:
                            # h1 d0:64 (v_rope) -> s4 rows 64:128
                            nc.vector.tensor_mul(
                                attnoutT[64:128, 4, :], at[0][0:64, :],
                                rb[0:64, :])
                            for c in range(4):
                                # s5+c rows 0:64  <- at[c] rows 64:128
                                nc.vector.tensor_mul(
                                    attnoutT[0:64, 5 + c, :],
                                    at[c][64:128, :], rb[64:128, :])
                                # s5+c rows 64:128 <- at[c+1] rows 0:64
                                nc.vector.tensor_mul(
                                    attnoutT[64:128, 5 + c, :],
                                    at[c + 1][0:64, :], rb[0:64, :])

                    if debug and b == 0 and blk == 1:
                        ncols = 2 * SBLK
                        nkt_d = 2 * SBLK // P
                        nc.gpsimd.memset(keyT[64:128, 4, 0:ncols], 0.0)
                        nc.gpsimd.memset(queryT[64:128, 4, :], 0.0)
                        nc.gpsimd.memset(queryT[64:128, 9, :], 0.0)
                        nc.gpsimd.dma_start(out=dbg_key[:, :, 0:ncols],
                                            in_=keyT[:, :, 0:ncols])
                        nc.gpsimd.dma_start(out=dbg_v[:, 0:nkt_d, :],
                                            in_=V[:, 0:nkt_d, 0:577])
                        nc.gpsimd.dma_start(out=dbg_q[:, :, :],
                                            in_=queryT[:, :, :])
                        nc.gpsimd.dma_start(out=dbg_at[:, :, :],
                                            in_=attnoutT[:, :, :])

                    # ---- out = attnoutT^T @ w_oT (rows already
                    # normalized; heads mix freely) ----
                    for ct in range(HID // SBLK if stage >= 3 else 0):
                        for qs in range(4):
                            ps = psQ.tile([P, SBLK], fp32, tag="psQ",
                                          name="psO")
                            for s_i in range(9):
                                nc.tensor.matmul(
                                    ps[:, :],
                                    attnoutT[:, s_i, qs * P:(qs + 1) * P],
                                    woT[:, s_i, ct * SBLK:(ct + 1) * SBLK],
                                    start=(s_i == 0), stop=(s_i == 8))
                            osb = work.tile([P, SBLK], bf16, tag="osb",
                                            bufs=3, name="osb")
                            if (ct + qs) % 2 == 0:
                                nc.vector.tensor_copy(out=osb[:, :],
                                                      in_=ps[:, :])
                            else:
                                nc.scalar.copy(out=osb[:, :], in_=ps[:, :])
                            nc.gpsimd.dma_start(
                                out=out_d[rows0 + qs * P:
                                          rows0 + (qs + 1) * P,
                                          ct * SBLK:(ct + 1) * SBLK],
                                in_=osb[:, :])

    nc.compile()
    return nc


def make_in_maps(inputs, b_count=B, s_len=S):
    hidden = np.asarray(inputs["hidden_states"],
                        dtype=np.float32).reshape(b_count * s_len, HID)
    cos = np.asarray(inputs["cos"], dtype=np.float32)[0, :s_len]  # [s,64]
    sin = np.asarray(inputs["sin"], dtype=np.float32)[0, :s_len]
    w_qa = np.asarray(inputs["w_qa"], np.float32)
    w_qb = np.asarray(inputs["w_qb"], np.float32)
    w_kv = np.asarray(inputs["w_kv"], np.float32)
    w_o = np.asarray(inputs["w_o"], np.float32)

    hidT = np.ascontiguousarray(hidden.T).astype(BF)            # [HID, R]
    wkvT = np.ascontiguousarray(w_kv.T).astype(BF)              # [HID, 640]
    W_full = w_qb @ w_qa                                        # [H*576, HID]

    # rope table: rows 0:64 cos^T; rows 64:96 sin^T[32:64]; 96:128 sin^T[0:32]
    ropeT = np.ascontiguousarray(np.concatenate(
        [cos.T, sin.T[32:64], sin.T[0:32]], axis=0)).astype(BF)  # [128, s]

    r = np.arange(P)[:, None]
    q = np.arange(P)[None, :]
    maskd = (r <= q).astype(BF)                                 # [128,128]

    in_maps = []
    for c in range(N_CORES):
        W_h = W_full[c * OC:(c + 1) * OC]                       # [1152, HID]
        whT = np.ascontiguousarray(W_h.T).astype(BF)            # [HID, 1152]
        woT = np.ascontiguousarray(
            w_o[:, c * OC:(c + 1) * OC].T).astype(BF)           # [1152, HID]
        in_maps.append({
            "hidT": hidT,
            "whT": whT,
            "wkvT": wkvT,
            "woT": woT,
            "ropeT": ropeT,
            "maskd": maskd,
        })
    return in_maps


_NC_CACHE = {}


def run_on_hw(inputs, trace=False):
    import os

    from concourse.bass_utils import run_bass_kernel_spmd

    if not trace:
        os.environ["BASS_NEVER_TRACE"] = "1"

    key = "full"
    if key not in _NC_CACHE:
        _NC_CACHE[key] = build_nc()
    nc = _NC_CACHE[key]
    in_maps = make_in_maps(inputs)
    res = run_bass_kernel_spmd(nc, in_maps, core_ids=list(range(N_CORES)),
                               trace=trace)
    acc = np.zeros((B * S, HID), dtype=np.float32)
    for r in res.results:
        acc += r["out_part"]
    return acc.reshape(B, S, HID), res


def kernel(**inputs):
    out, _ = run_on_hw(inputs, trace=False)
    return out
